# revision 11
# baseline (speedup 1.0000x reference)
"""Trainium2 Bass kernel for BondCenteredTensorMomentDescriptor.

Per-edge pipeline (fully edge-parallel, sharded across 8 NeuronCores):
  a1 = atoms[i[e]], a2 = atoms[j[e]]               (indirect-DMA gather)
  y   = CG_tensor_product(a1, a2, w1)              (DVE sparse bilinear)
  bexp = sph(disp/r) * sinc-radial * cosine-cutoff (DVE/ACT)
  out = CG_tensor_product(y, bexp, w2)             (DVE sparse bilinear)

Layout on chip: "pure-edge" tiles [128 partitions = 128 edge-rows,
free = (comp 25, chan 16, g Gp)] so each DVE instruction covers
128*Gp edges with free-dim 16*Gp.
"""

import sys
import os
import numpy as np
from math import factorial, sqrt, pi

for _p in ("/opt/trn_rl_repo",):
    if _p not in sys.path:
        sys.path.insert(0, _p)

import concourse.bass as bass
import concourse.bacc as bacc
import concourse.tile as tile
from concourse import mybir

f32 = mybir.dt.float32
i32 = mybir.dt.int32
ALU = mybir.AluOpType
ACTF = mybir.ActivationFunctionType

MAX_DEG = 4
CUTOFF = 5.0
P = 128

# ---------------------------------------------------------------------------
# Clebsch-Gordan tables (compile-time constants; mirrors the reference math)
# ---------------------------------------------------------------------------


def _cg_complex(j1, m1, j2, m2, j3, m3):
    if m1 + m2 != m3 or j3 < abs(j1 - j2) or j3 > j1 + j2:
        return 0.0
    f = factorial
    pre = sqrt(
        (2 * j3 + 1) * f(j1 + j2 - j3) * f(j1 - j2 + j3) * f(-j1 + j2 + j3)
        / f(j1 + j2 + j3 + 1)
    )
    pre *= sqrt(f(j1 + m1) * f(j1 - m1) * f(j2 + m2) * f(j2 - m2) * f(j3 + m3) * f(j3 - m3))
    s = 0.0
    for k in range(0, j1 + j2 + j3 + 1):
        t = [k, j1 + j2 - j3 - k, j1 - m1 - k, j2 + m2 - k, j3 - j2 + m1 + k, j3 - j1 - m2 + k]
        if min(t) < 0:
            continue
        s += (-1) ** k / (f(t[0]) * f(t[1]) * f(t[2]) * f(t[3]) * f(t[4]) * f(t[5]))
    return pre * s


def _u_matrix(l):
    U = np.zeros((2 * l + 1, 2 * l + 1), dtype=np.complex128)
    U[l, l] = 1.0
    inv_sqrt2 = 1.0 / sqrt(2.0)
    for m in range(1, l + 1):
        U[l + m, l + m] = ((-1) ** m) * inv_sqrt2
        U[l + m, l - m] = inv_sqrt2
        U[l - m, l - m] = 1j * inv_sqrt2
        U[l - m, l + m] = -1j * ((-1) ** m) * inv_sqrt2
    return U


def _real_cg(l1, l2, l3):
    C = np.zeros((2 * l1 + 1, 2 * l2 + 1, 2 * l3 + 1), dtype=np.complex128)
    for m1 in range(-l1, l1 + 1):
        for m2 in range(-l2, l2 + 1):
            m3 = m1 + m2
            if -l3 <= m3 <= l3:
                C[m1 + l1, m2 + l2, m3 + l3] = _cg_complex(l1, m1, l2, m2, l3, m3)
    R = np.einsum("ai,bj,ck,ijk->abc", _u_matrix(l1), _u_matrix(l2), _u_matrix(l3).conj(), C)
    return np.real(R).astype(np.float32)


def _build_cg_paths():
    paths = []
    for l1 in range(MAX_DEG + 1):
        for l2 in range(MAX_DEG + 1):
            for l3 in range(abs(l1 - l2), min(MAX_DEG, l1 + l2) + 1):
                if (l1 + l2 + l3) % 2 == 0:
                    paths.append((l1, l2, l3, _real_cg(l1, l2, l3)))
    return paths


CG_PATHS = _build_cg_paths()
NUM_PATHS = len(CG_PATHS)  # 42


def _build_tp_plan():
    """Per path: (l1, l2, l3, [(i, [(j, [(k, cg)])])]) with nonzero cg only."""
    plan = []
    for pidx, (l1, l2, l3, cg) in enumerate(CG_PATHS):
        ilist = []
        for i in range(2 * l1 + 1):
            jlist = []
            for j in range(2 * l2 + 1):
                klist = [(k, float(cg[i, j, k])) for k in range(2 * l3 + 1)
                         if abs(cg[i, j, k]) > 1e-10]
                if klist:
                    jlist.append((j, klist))
            if jlist:
                ilist.append((i, jlist))
        plan.append((pidx, l1, l2, l3, ilist))
    return plan


TP_PLAN = _build_tp_plan()

# ---------------------------------------------------------------------------
# Real spherical harmonics as polynomials: plan of z-polynomials + Cm/Sm
# ---------------------------------------------------------------------------


def _sph_poly_plan():
    """Returns dict (l, m) -> np coeff array c[p] for scaled Legendre part
    Ptil_lm(z) = scale_lm * Q_lm(z), highest degree coeff last.
    scale_lm = sqrt((2l+1)/4pi) for m = 0, sqrt(2) * K_lm otherwise.
    """
    L = MAX_DEG
    z = np.polynomial.polynomial.Polynomial([0.0, 1.0])
    one = np.polynomial.polynomial.Polynomial([1.0])
    Q = {(0, 0): one}
    for m in range(L + 1):
        if m > 0:
            Q[(m, m)] = (2 * m - 1) * Q[(m - 1, m - 1)]
        if m + 1 <= L:
            Q[(m + 1, m)] = (2 * m + 1) * z * Q[(m, m)]
        for l in range(m + 2, L + 1):
            Q[(l, m)] = ((2 * l - 1) * z * Q[(l - 1, m)] - (l + m - 1) * Q[(l - 2, m)]) / (l - m)
    out = {}
    for l in range(L + 1):
        for m in range(l + 1):
            if m == 0:
                scale = sqrt((2 * l + 1) / (4 * pi))
            else:
                Klm = sqrt((2 * l + 1) / (4 * pi) * factorial(l - m) / factorial(l + m))
                scale = sqrt(2.0) * Klm
            out[(l, m)] = (scale * Q[(l, m)]).coef.astype(np.float64)
    return out


SPH_POLYS = _sph_poly_plan()

# ---------------------------------------------------------------------------
# Device kernel builder
# ---------------------------------------------------------------------------


def _emit_sph(nc, sp, u_t, y_t, Gp):
    """u_t: [P,3,Gp] unit vectors -> y_t: [P,25,Gp] real spherical harmonics."""
    ux, uy, uz = u_t[:, 0], u_t[:, 1], u_t[:, 2]
    L = MAX_DEG

    # z powers: zp[1] = z, zp[2] = z^2, ...
    zpow = {1: uz}
    z2 = sp.tile([P, Gp], f32, tag="z2")
    nc.vector.tensor_tensor(out=z2[:], in0=uz, in1=uz, op=ALU.mult)
    zpow[2] = z2[:]
    z3 = sp.tile([P, Gp], f32, tag="z3")
    nc.vector.tensor_tensor(out=z3[:], in0=z2[:], in1=uz, op=ALU.mult)
    zpow[3] = z3[:]
    z4 = sp.tile([P, Gp], f32, tag="z4")
    nc.vector.tensor_tensor(out=z4[:], in0=z2[:], in1=z2[:], op=ALU.mult)
    zpow[4] = z4[:]

    # Cm/Sm recurrences (C1 = x, S1 = y used in place)
    Cm = {1: ux}
    Sm = {1: uy}
    for m in range(2, L + 1):
        c = sp.tile([P, Gp], f32, tag=f"c{m}")
        s = sp.tile([P, Gp], f32, tag=f"s{m}")
        t1 = sp.tile([P, Gp], f32, tag="cmt1")
        t2 = sp.tile([P, Gp], f32, tag="cmt2")
        nc.vector.tensor_tensor(out=t1[:], in0=ux, in1=Cm[m - 1], op=ALU.mult)
        nc.vector.tensor_tensor(out=t2[:], in0=uy, in1=Sm[m - 1], op=ALU.mult)
        nc.vector.tensor_tensor(out=c[:], in0=t1[:], in1=t2[:], op=ALU.subtract)
        t3 = sp.tile([P, Gp], f32, tag="cmt3")
        t4 = sp.tile([P, Gp], f32, tag="cmt4")
        nc.vector.tensor_tensor(out=t3[:], in0=ux, in1=Sm[m - 1], op=ALU.mult)
        nc.vector.tensor_tensor(out=t4[:], in0=uy, in1=Cm[m - 1], op=ALU.mult)
        nc.vector.tensor_tensor(out=s[:], in0=t3[:], in1=t4[:], op=ALU.add)
        Cm[m] = c[:]
        Sm[m] = s[:]

    def emit_poly(dst, coef):
        """dst[:] = sum_p coef[p] * z^p (polynomial evaluation)."""
        nz = [(p, float(c)) for p, c in enumerate(coef) if abs(c) > 1e-14]
        first = True
        const_term = 0.0
        for (p, c) in nz:
            if p == 0:
                const_term = c
                continue
            if first:
                nc.vector.tensor_scalar(out=dst, in0=zpow[p], scalar1=c,
                                        scalar2=None, op0=ALU.mult)
                first = False
            else:
                nc.vector.scalar_tensor_tensor(out=dst, in0=zpow[p], scalar=c,
                                               in1=dst, op0=ALU.mult, op1=ALU.add)
        if first:
            # constant polynomial
            nc.vector.memset(dst, const_term)
        elif const_term != 0.0:
            nc.vector.tensor_scalar(out=dst, in0=dst, scalar1=const_term,
                                    scalar2=None, op0=ALU.add)

    for l in range(L + 1):
        # m = 0 term goes straight into its slot
        emit_poly(y_t[:, l * l + l], SPH_POLYS[(l, 0)])
        for m in range(1, l + 1):
            base = sp.tile([P, Gp], f32, tag="base")
            emit_poly(base[:], SPH_POLYS[(l, m)])
            nc.vector.tensor_tensor(out=y_t[:, l * l + l + m], in0=base[:],
                                    in1=Cm[m], op=ALU.mult)
            nc.vector.tensor_tensor(out=y_t[:, l * l + l - m], in0=base[:],
                                    in1=Sm[m], op=ALU.mult)


def _emit_tp(nc, fold_pool, pair_pool, src_t, w_t, other, out_t, Gp, second_tp):
    """Emit one CG tensor product.

    src_t:  [P,25,16,Gp]  (a1 for TP1, y for TP2)
    w_t:    [P,42,16]     per-path weights (replicated across partitions)
    other:  a2_t [P,25,16,Gp] (TP1) or Y_t [P,25,Gp] (TP2, bcast over chan)
    out_t:  [P,25,16,Gp]
    """
    written = set()
    for (pidx, l1, l2, l3, ilist) in TP_PLAN:
        wb = w_t[:, pidx, :].unsqueeze(2).to_broadcast([P, 16, Gp])
        for (i, jlist) in ilist:
            fold = fold_pool.tile([P, 16, Gp], f32, tag="fold")
            nc.vector.tensor_tensor(out=fold[:], in0=src_t[:, l1 * l1 + i],
                                    in1=wb, op=ALU.mult)
            for (j, klist) in jlist:
                if second_tp:
                    in1 = other[:, l2 * l2 + j].unsqueeze(1).to_broadcast([P, 16, Gp])
                else:
                    in1 = other[:, l2 * l2 + j]
                pp = pair_pool.tile([P, 16, Gp], f32, tag="pp")
                nc.vector.tensor_tensor(out=pp[:], in0=fold[:], in1=in1, op=ALU.mult)
                for (k, cgv) in klist:
                    ko = l3 * l3 + k
                    if ko not in written:
                        nc.vector.tensor_scalar(out=out_t[:, ko], in0=pp[:],
                                                scalar1=cgv, scalar2=None,
                                                op0=ALU.mult)
                        written.add(ko)
                    else:
                        nc.vector.scalar_tensor_tensor(out=out_t[:, ko], in0=pp[:],
                                                       scalar=cgv, in1=out_t[:, ko],
                                                       op0=ALU.mult, op1=ALU.add)


def build_nc(natoms, T, Gp, debug=False):
    """Build the per-core Bass program. Edges per core = T * 128 * Gp."""
    nc = bacc.Bacc("TRN2", target_bir_lowering=False)
    CF = 25 * 16

    atoms_d = nc.dram_tensor("atoms", [natoms, CF], f32, kind="ExternalInput")
    idxi_d = nc.dram_tensor("idxi", [T, P, Gp], i32, kind="ExternalInput")
    idxj_d = nc.dram_tensor("idxj", [T, P, Gp], i32, kind="ExternalInput")
    disp_d = nc.dram_tensor("disp", [T, P, 3 * Gp], f32, kind="ExternalInput")
    w1_d = nc.dram_tensor("w1bc", [P, NUM_PATHS * 16], f32, kind="ExternalInput")
    w2_d = nc.dram_tensor("w2bc", [P, NUM_PATHS * 16], f32, kind="ExternalInput")
    ktab_d = nc.dram_tensor("ktab", [P, 16], f32, kind="ExternalInput")
    out_d = nc.dram_tensor("out", [T, P, 25 * 16 * Gp], f32, kind="ExternalOutput")
    if debug:
        dbg_a1 = nc.dram_tensor("dbg_a1", [T, P, 25 * 16 * Gp], f32, kind="ExternalOutput")
        dbg_y = nc.dram_tensor("dbg_y", [T, P, 25 * 16 * Gp], f32, kind="ExternalOutput")
        dbg_ysph = nc.dram_tensor("dbg_ysph", [T, P, 25 * Gp], f32, kind="ExternalOutput")
        dbg_radc = nc.dram_tensor("dbg_radc", [T, P, 16 * Gp], f32, kind="ExternalOutput")

    with tile.TileContext(nc) as tc:
        with (
            tc.tile_pool(name="consts", bufs=1) as consts,
            tc.tile_pool(name="big", bufs=3) as big_pool,
            tc.tile_pool(name="fold", bufs=3) as fold_pool,
            tc.tile_pool(name="pair", bufs=4) as pair_pool,
            tc.tile_pool(name="small", bufs=1) as small_pool,
            tc.tile_pool(name="sph", bufs=1) as sph_pool,
        ):
            stage_pool = big_pool
            yout_pool = big_pool
            w1_t = consts.tile([P, NUM_PATHS, 16], f32)
            nc.sync.dma_start(out=w1_t[:].rearrange("p a b -> p (a b)"), in_=w1_d[:, :])
            w2_t = consts.tile([P, NUM_PATHS, 16], f32)
            nc.sync.dma_start(out=w2_t[:].rearrange("p a b -> p (a b)"), in_=w2_d[:, :])
            ktab_t = consts.tile([P, 16], f32)
            nc.sync.dma_start(out=ktab_t[:], in_=ktab_d[:, :])
            neg_half_pi = consts.tile([P, 1], f32)
            nc.vector.memset(neg_half_pi[:], -pi / 2.0)

            for t in range(T):
                # ---- gather a1, a2 ----
                idxi_t = small_pool.tile([P, Gp], i32, tag="idxi")
                nc.sync.dma_start(out=idxi_t[:], in_=idxi_d[t])
                idxj_t = small_pool.tile([P, Gp], i32, tag="idxj")
                nc.sync.dma_start(out=idxj_t[:], in_=idxj_d[t])

                a_ts = []
                for idx_t in (idxi_t, idxj_t):
                    stage = big_pool.tile([P, Gp, 25, 16], f32, tag="big")
                    a_t = big_pool.tile([P, 25, 16, Gp], f32, tag="big")
                    a_ts.append(a_t)
                    # HW indirect DMA applies one index per partition, so
                    # issue Gp gathers of 128 rows each.
                    for g in range(Gp):
                        nc.gpsimd.indirect_dma_start(
                            out=stage[:, g].rearrange("p c f -> p (c f)"),
                            out_offset=None,
                            in_=atoms_d[:, :],
                            in_offset=bass.IndirectOffsetOnAxis(
                                ap=idx_t[:, g:g + 1], axis=0),
                        )
                    # permute (g,c,f) -> (c,f,g)
                    nc.scalar.copy(out=a_t[:].transpose([0, 3, 1, 2]), in_=stage[:])
                a1_t, a2_t = a_ts

                # ---- bond geometry ----
                disp_t = small_pool.tile([P, 3, Gp], f32, tag="disp")
                nc.sync.dma_start(out=disp_t[:].rearrange("p c g -> p (c g)"),
                                  in_=disp_d[t])
                sq = small_pool.tile([P, 3, Gp], f32, tag="sq")
                nc.vector.tensor_tensor(out=sq[:], in0=disp_t[:], in1=disp_t[:], op=ALU.mult)
                r2 = small_pool.tile([P, Gp], f32, tag="r2")
                nc.vector.tensor_tensor(out=r2[:], in0=sq[:, 0], in1=sq[:, 1], op=ALU.add)
                nc.vector.tensor_tensor(out=r2[:], in0=r2[:], in1=sq[:, 2], op=ALU.add)
                nc.vector.tensor_scalar(out=r2[:], in0=r2[:], scalar1=1e-12,
                                        scalar2=None, op0=ALU.add)
                r_t = small_pool.tile([P, Gp], f32, tag="r")
                nc.scalar.sqrt(out=r_t[:], in_=r2[:])
                rinv = small_pool.tile([P, Gp], f32, tag="rinv")
                nc.vector.reciprocal(out=rinv[:], in_=r_t[:])
                u_t = small_pool.tile([P, 3, Gp], f32, tag="u")
                nc.vector.tensor_tensor(
                    out=u_t[:], in0=disp_t[:],
                    in1=rinv[:].unsqueeze(1).to_broadcast([P, 3, Gp]), op=ALU.mult)

                # ---- spherical harmonics of u ----
                ysph_t = small_pool.tile([P, 25, Gp], f32, tag="ysph")
                _emit_sph(nc, sph_pool, u_t, ysph_t, Gp)

                # ---- radial basis * cutoff:
                # radc[f,g] = sinc(rc*k[f]/C) * 0.5*(1+cos(pi*rc/C)), rc = min(r, C)
                rc = small_pool.tile([P, Gp], f32, tag="rc")
                nc.vector.tensor_scalar(out=rc[:], in0=r_t[:], scalar1=CUTOFF,
                                        scalar2=None, op0=ALU.min)
                rkf = small_pool.tile([P, 16, Gp], f32, tag="rkf")
                nc.vector.tensor_tensor(
                    out=rkf[:], in0=rc[:].unsqueeze(1).to_broadcast([P, 16, Gp]),
                    in1=ktab_t[:].unsqueeze(2).to_broadcast([P, 16, Gp]), op=ALU.mult)
                # range-reduce rkf to tmod in [-1, 1] with tmod = rkf (mod 2):
                # tmod = rkf - 2*int(rkf/2), then fold once each way (works for
                # any cast rounding mode).
                half = small_pool.tile([P, 16, Gp], f32, tag="half")
                nc.vector.tensor_scalar(out=half[:], in0=rkf[:], scalar1=0.5,
                                        scalar2=None, op0=ALU.mult)
                hint = small_pool.tile([P, 16, Gp], i32, tag="hint")
                nc.vector.tensor_copy(out=hint[:], in_=half[:])
                hflt = small_pool.tile([P, 16, Gp], f32, tag="hflt")
                nc.vector.tensor_copy(out=hflt[:], in_=hint[:])
                tmod = small_pool.tile([P, 16, Gp], f32, tag="tmod")
                nc.vector.scalar_tensor_tensor(out=tmod[:], in0=hflt[:], scalar=-2.0,
                                               in1=rkf[:], op0=ALU.mult, op1=ALU.add)
                tge = small_pool.tile([P, 16, Gp], f32, tag="tge")
                nc.vector.tensor_scalar(out=tge[:], in0=tmod[:], scalar1=1.0,
                                        scalar2=None, op0=ALU.is_gt)
                nc.vector.scalar_tensor_tensor(out=tmod[:], in0=tge[:], scalar=-2.0,
                                               in1=tmod[:], op0=ALU.mult, op1=ALU.add)
                nc.vector.tensor_scalar(out=tge[:], in0=tmod[:], scalar1=-1.0,
                                        scalar2=None, op0=ALU.is_lt)
                nc.vector.scalar_tensor_tensor(out=tmod[:], in0=tge[:], scalar=2.0,
                                               in1=tmod[:], op0=ALU.mult, op1=ALU.add)
                sinv = small_pool.tile([P, 16, Gp], f32, tag="sinv")
                nc.scalar.activation(out=sinv[:], in_=tmod[:], func=ACTF.Sin,
                                     scale=pi)
                rec = small_pool.tile([P, 16, Gp], f32, tag="rec")
                nc.vector.reciprocal(out=rec[:], in_=rkf[:])
                radc = small_pool.tile([P, 16, Gp], f32, tag="radc")
                nc.vector.tensor_tensor(out=radc[:], in0=sinv[:], in1=rec[:], op=ALU.mult)
                # cutoff (and absorb the 1/pi from sinc): cutv = (0.5 - 0.5*sin(pi*rc/C - pi/2)) / pi
                scut = small_pool.tile([P, Gp], f32, tag="scut")
                nc.scalar.activation(out=scut[:], in_=rc[:], func=ACTF.Sin,
                                     scale=pi / CUTOFF, bias=neg_half_pi[:, :1])
                cutv = small_pool.tile([P, Gp], f32, tag="cutv")
                nc.vector.tensor_scalar(out=cutv[:], in0=scut[:], scalar1=-0.5 / pi,
                                        scalar2=0.5 / pi, op0=ALU.mult, op1=ALU.add)
                nc.vector.tensor_tensor(
                    out=radc[:], in0=radc[:],
                    in1=cutv[:].unsqueeze(1).to_broadcast([P, 16, Gp]), op=ALU.mult)

                # ---- TP1: y = tp(a1, a2, w1) ----
                y_t = yout_pool.tile([P, 25, 16, Gp], f32, tag="big")
                _emit_tp(nc, fold_pool, pair_pool, a1_t, w1_t, a2_t, y_t, Gp,
                         second_tp=False)

                if debug:
                    nc.sync.dma_start(out=dbg_a1[t], in_=a1_t[:].rearrange("p c f g -> p (c f g)"))
                    nc.sync.dma_start(out=dbg_y[t], in_=y_t[:].rearrange("p c f g -> p (c f g)"))
                    nc.sync.dma_start(out=dbg_ysph[t], in_=ysph_t[:].rearrange("p c g -> p (c g)"))
                    nc.sync.dma_start(out=dbg_radc[t], in_=radc[:].rearrange("p f g -> p (f g)"))
                # ---- TP2: out = tp(y, ysph, w2) * radc ----
                out_t = yout_pool.tile([P, 25, 16, Gp], f32, tag="big")
                _emit_tp(nc, fold_pool, pair_pool, y_t, w2_t, ysph_t, out_t, Gp,
                         second_tp=True)
                nc.vector.tensor_tensor(
                    out=out_t[:], in0=out_t[:],
                    in1=radc[:].unsqueeze(1).to_broadcast([P, 25, 16, Gp]), op=ALU.mult)

                nc.sync.dma_start(out=out_d[t],
                                  in_=out_t[:].rearrange("p c f g -> p (c f g)"))
    nc.finalize()
    return nc


# ---------------------------------------------------------------------------
# Host glue
# ---------------------------------------------------------------------------

_NC_CACHE = {}
GP_DEFAULT = 32


def _get_nc(natoms, T, Gp):
    key = (natoms, T, Gp)
    if key not in _NC_CACHE:
        _NC_CACHE[key] = build_nc(natoms, T, Gp)
    return _NC_CACHE[key]


def make_in_maps(atomic_descriptors, neighbour_displacements, weights1, weights2,
                 neighbours_i, neighbours_j, n_cores, T, Gp):
    N = atomic_descriptors.shape[0]
    E = neighbours_i.shape[0]
    F = atomic_descriptors.shape[-1]
    assert F == 16 and atomic_descriptors.shape[2] == 25
    ecore = (E + n_cores - 1) // n_cores
    epad = T * P * Gp
    assert epad >= ecore

    atoms = np.ascontiguousarray(atomic_descriptors.reshape(N, 25 * 16), dtype=np.float32)
    w1bc = np.ascontiguousarray(
        np.broadcast_to(weights1.reshape(-1)[None, :], (P, NUM_PATHS * 16)), dtype=np.float32)
    w2bc = np.ascontiguousarray(
        np.broadcast_to(weights2.reshape(-1)[None, :], (P, NUM_PATHS * 16)), dtype=np.float32)
    ktab = np.ascontiguousarray(
        np.broadcast_to((np.arange(1, 17, dtype=np.float32) / CUTOFF)[None, :], (P, 16)))

    in_maps = []
    for c in range(n_cores):
        lo = c * ecore
        hi = min(E, lo + ecore)
        n = hi - lo
        ii = np.zeros(epad, dtype=np.int32)
        jj = np.zeros(epad, dtype=np.int32)
        dd = np.zeros((epad, 3), dtype=np.float32)
        dd[:, 0] = 1.0
        ii[:n] = neighbours_i[lo:hi]
        jj[:n] = neighbours_j[lo:hi]
        dd[:n] = neighbour_displacements[lo:hi]
        in_maps.append({
            "atoms": atoms,
            "idxi": ii.reshape(T, P, Gp),
            "idxj": jj.reshape(T, P, Gp),
            "disp": np.ascontiguousarray(
                dd.reshape(T, P, Gp, 3).transpose(0, 1, 3, 2).reshape(T, P, 3 * Gp)),
            "w1bc": w1bc,
            "w2bc": w2bc,
            "ktab": ktab,
        })
    return in_maps


def unpack_outputs(results, E, n_cores, T, Gp):
    ecore = (E + n_cores - 1) // n_cores
    parts = []
    for c in range(n_cores):
        o = results[c]["out"].reshape(T, P, 25, 16, Gp)
        o = o.transpose(0, 1, 4, 2, 3).reshape(T * P * Gp, 25, 16)
        lo = c * ecore
        hi = min(E, lo + ecore)
        parts.append(o[: hi - lo])
    out = np.concatenate(parts, axis=0)
    return out.reshape(E, 1, 25, 16).astype(np.float32)


def kernel(atomic_descriptors, neighbour_displacements, weights1, weights2,
           neighbours_i, neighbours_j):
    from concourse import bass_utils

    N_CORES = 8
    Gp = GP_DEFAULT
    E = neighbours_i.shape[0]
    ecore = (E + N_CORES - 1) // N_CORES
    T = (ecore + P * Gp - 1) // (P * Gp)
    natoms = atomic_descriptors.shape[0]

    nc = _get_nc(natoms, T, Gp)
    in_maps = make_in_maps(
        np.asarray(atomic_descriptors), np.asarray(neighbour_displacements),
        np.asarray(weights1), np.asarray(weights2),
        np.asarray(neighbours_i), np.asarray(neighbours_j),
        N_CORES, T, Gp)
    res = bass_utils.run_bass_kernel_spmd(nc, in_maps, core_ids=list(range(N_CORES)))
    return unpack_outputs(res.results, E, N_CORES, T, Gp)
